# revision 49
# baseline (speedup 1.0000x reference)
"""DeepseekV2 MLA attention forward — Trainium2 Bass kernel (8 NeuronCores).

Sharding: data-parallel over batch (2) x sequence-panel-parallel over query
rows (4 panels of 512) = 8 cores. Each core computes, for its (batch, panel):
  - q path (q_a_proj -> rmsnorm -> q_b_proj) for its 512 query rows, all heads
  - kv path (kv_a_proj -> rmsnorm -> RoPE) for its OWN 512 keys, then a
    4-core AllGather over the panel group assembles the full-S compressed kv
  - kv_b expansion, full attention (16 heads) for its query rows, o_proj
Output panels are concatenated on the host.

v3 vs v2:
  - every DRAM tensor is pre-laid-out on the host so each dma_start moves
    large contiguous runs per partition (big descriptors, no rearranges)
  - rmsnorm sum-of-squares via DVE square/accumulate + GpSimd partition
    reduce: the PE stream in phases A/B is pure GEMMs (no norm bubbles)
  - kv_a computed for S/4 keys per core + AllGather (was 4x redundant)
  - PSUM->SBUF copies of V / k_nope moved from Scalar (near-critical) to DVE
Everything on-chip is fp16 except fp32 PSUM accumulation and norm/rope math.
"""

import os
import numpy as np
from contextlib import ExitStack

import concourse.bass as bass
import concourse.bacc as bacc
import concourse.mybir as mybir
import concourse.tile as tile
from concourse import bass_utils
from concourse.bass_isa import ReduceOp



B, S, HID = 2, 2048, 2048
NH = 16
QLR, KVLR = 1536, 512
DN, DR, DV = 128, 64, 128
DQK = DN + DR
SCALE = DQK ** -0.5
EPS = 1e-6
P = 128
NPANEL = 4
W = S // NPANEL            # 512 query rows per core
NCORES = B * NPANEL

F32 = mybir.dt.float32
F16 = mybir.dt.float16
EXP = mybir.ActivationFunctionType.Exp
SQRT = mybir.ActivationFunctionType.Sqrt
COPY = mybir.ActivationFunctionType.Copy
MULT = mybir.AluOpType.mult
ADD = mybir.AluOpType.add

KB_HID = HID // P          # 16
KB_QLR = QLR // P          # 12
KB_CKV = KVLR // P         # 4
KB_S = S // P              # 16
MB_QLR = QLR // P          # 12
MB_NOPE = NH * DN // P     # 16
MB_PE = NH * DR // P       # 8
MB_HID = HID // P          # 16
NG = NH // 2               # head-pair groups

LAST_RESULT = None         # BassKernelResults of the most recent launch


def _mm(nc, out, lhsT, rhs, start, stop):
    nc.tensor.matmul(out, lhsT, rhs, start=start, stop=stop)


def _emit(tc, t, with_mask):
    """Emit the whole per-core program. `t` maps tensor name -> DRAM AP."""
    nc = tc.nc

    with ExitStack() as big:
        const = big.enter_context(tc.tile_pool(name="const", bufs=1))
        qa_ln = const.tile([P, KB_QLR], F32)
        nc.sync.dma_start(qa_ln[:], t["qa_ln_p"][:])
        kva_ln = const.tile([P, KB_CKV], F32)
        nc.sync.dma_start(kva_ln[:], t["kva_ln_p"][:])
        cos2p = const.tile([P, W], F32)
        nc.sync.dma_start(cos2p[:], t["cos2p"][:])
        sin2sp = const.tile([P, W], F32)
        nc.sync.dma_start(sin2sp[:], t["sin2sp"][:])
        epsP = const.tile([P, 1], F32)
        nc.vector.memset(epsP[:], EPS)

        # resident activations (allocated before the A/B pool; LIFO free)
        qa_pool = big.enter_context(tc.tile_pool(name="qaT", bufs=1))
        qaT = qa_pool.tile([P, KB_QLR, W], F16)          # 12 KB/part
        ckv_pool = big.enter_context(tc.tile_pool(name="ckv", bufs=1))
        ckT = ckv_pool.tile([P, KB_CKV, S], F16)         # 16 KB/part
        kpe2 = ckv_pool.tile([P, S], F16)                # 4 KB/part
        dram = big.enter_context(tc.tile_pool(name="ccd", bufs=1,
                                              space="DRAM"))
        cc_in = dram.tile([P, KB_CKV + 1, W], F16)
        cc_out = dram.tile([NPANEL, P, KB_CKV + 1, W], F16)

        def rms_inv(pool, sqacc, inv_dim):
            """[P,W] broadcast 1/sqrt(mean+eps) from f16 partial squares."""
            sums = pool.tile([P, W], F32, tag="sums")
            nc.gpsimd.partition_all_reduce(sums[:], sqacc[:], P, ReduceOp.add)
            srt = pool.tile([P, W], F32, tag="srt")
            nc.scalar.activation(srt[:], sums[:], SQRT, bias=epsP[:],
                                 scale=inv_dim)
            rq = pool.tile([P, W], F32, tag="rq")
            nc.vector.reciprocal(rq[:], srt[:])
            return rq

        # ---------------- phases A+B (panel chunk of hidden) ---------------
        with tc.tile_pool(name="ab", bufs=1) as ab_pool:
            hp = ab_pool.tile([P, KB_HID, W], F16)       # 16 KB/part
            nc.sync.dma_start(hp[:, :KB_HID // 2, :],
                              t["hsT_panel_p"][:, :KB_HID // 2, :])
            nc.scalar.dma_start(hp[:, KB_HID // 2:, :],
                                t["hsT_panel_p"][:, KB_HID // 2:, :])
            wkva = ab_pool.tile([P, KB_HID, KVLR + DR], F16)  # 18 KB/part
            # kpe columns first: phase B's m-loop starts with the kpe block
            nc.gpsimd.dma_start(wkva[:, :, KB_CKV * P:],
                                t["w_kva_p"][:, :, KB_CKV * P:])
            nc.gpsimd.dma_start(wkva[:, :, :KB_CKV * P],
                                t["w_kva_p"][:, :, :KB_CKV * P])
            # w_qa fully resident: its region must not alias phase B's
            # staging pools, and the loads overlap phase B's GEMMs
            wqa_all = ab_pool.tile([P, MB_QLR, KB_HID, P], F16)  # 48 KB/part
            for m in range(MB_QLR):
                eng = nc.sync if m % 2 == 0 else nc.scalar
                eng.dma_start(wqa_all[:, m, :, :], t["w_qa_p"][:, m, :, :])

            # ---- phases B then A in ONE scope: a single uninterrupted
            # PE stream of 17 GEMM chains; B first so the kv AllGather
            # launches early and overlaps the rest of A and phase C ----
            with tc.tile_pool(name="phAB", bufs=2) as pab, \
                 tc.tile_pool(name="phB_c", bufs=1) as pbc, \
                 tc.tile_pool(name="psA", bufs=3, space="PSUM") as psA:
                stg = pbc.tile([P, KB_CKV + 1, W], F16)  # 5 KB/part staging
                kpf = pbc.tile([P, W], F32)
                # ---- phase A chains (same scope, PE never goes cold) ----
                sqaccA = pab.tile([P, W], F16, tag="sqaccA")
                for m in range(MB_QLR):
                    ps = psA.tile([P, W], F32, tag="psA")
                    for k in range(KB_HID):
                        _mm(nc, ps[:], wqa_all[:, m, k, :], hp[:, k, :],
                            k == 0, k == KB_HID - 1)
                    nc.scalar.activation(qaT[:, m, :], ps[:], COPY)
                    sq = pab.tile([P, W], F16, tag="sq")
                    with nc.allow_low_precision(reason="fp16 sumsq"):
                        nc.vector.tensor_tensor(sq[:], qaT[:, m, :],
                                                qaT[:, m, :], MULT)
                        if m == 0:
                            nc.vector.tensor_copy(sqaccA[:], sq[:])
                        else:
                            nc.vector.tensor_tensor(sqaccA[:], sqaccA[:],
                                                    sq[:], ADD)
                rq = rms_inv(pab, sqaccA, 1.0 / QLR)
                for m in range(MB_QLR):
                    with nc.allow_low_precision(reason="fp16 activations"):
                        nc.vector.scalar_tensor_tensor(
                            qaT[:, m, :], qaT[:, m, :], qa_ln[:, m:m + 1],
                            rq[:], MULT, MULT)

                sqaccB = pab.tile([P, W], F16, tag="sqaccB")
                for m in (KB_CKV, 0, 1, 2, 3):   # kpe first: rope starts early
                    rows = P if m < KB_CKV else DR
                    ps = psA.tile([P, W], F32, tag="psA")
                    for k in range(KB_HID):
                        _mm(nc, ps[:rows, :], wkva[:, k, m * P:m * P + rows],
                            hp[:, k, :], k == 0, k == KB_HID - 1)
                    if m < KB_CKV:
                        nc.scalar.activation(stg[:, m, :], ps[:], COPY)
                        sq = pab.tile([P, W], F16, tag="sq")
                        with nc.allow_low_precision(reason="fp16 sumsq"):
                            nc.vector.tensor_tensor(sq[:], stg[:, m, :],
                                                    stg[:, m, :], MULT)
                            if m == 0:
                                nc.vector.tensor_copy(sqaccB[:], sq[:])
                            else:
                                nc.vector.tensor_tensor(sqaccB[:], sqaccB[:],
                                                        sq[:], ADD)
                    else:
                        nc.scalar.activation(kpf[0:DR, :], ps[0:DR, :], COPY)
                        nc.vector.tensor_copy(kpf[DR:P, :], ps[0:DR, :])
                        # rope immediately (overlaps the ckv GEMMs)
                        rot = pbc.tile([P, W], F32, tag="rot")
                        for hh in (0, DR):
                            nc.vector.tensor_copy(rot[hh:hh + 32, :],
                                                  kpf[hh + 32:hh + 64, :])
                            nc.vector.tensor_copy(rot[hh + 32:hh + 64, :],
                                                  kpf[hh:hh + 32, :])
                        nc.vector.tensor_tensor(kpf[:], kpf[:], cos2p[:],
                                                MULT)
                        nc.vector.tensor_tensor(rot[:], rot[:], sin2sp[:],
                                                MULT)
                        with nc.allow_low_precision(reason="fp16 act"):
                            nc.vector.tensor_tensor(stg[:, KB_CKV, :],
                                                    kpf[:], rot[:], ADD)
                rk = rms_inv(pab, sqaccB, 1.0 / KVLR)
                for m in range(KB_CKV):
                    with nc.allow_low_precision(reason="fp16 activations"):
                        nc.vector.scalar_tensor_tensor(
                            stg[:, m, :], stg[:, m, :],
                            kva_ln[:, m:m + 1], rk[:], MULT, MULT)
                # all-gather the compressed kv across the 4 panel cores
                # (gpsimd issue queue: keeps the collective's semaphore
                # waits off the sync engine so phase C DMAs aren't blocked)
                nc.gpsimd.dma_start(cc_in[:], stg[:])
                nc.gpsimd.collective_compute(
                    "AllGather", mybir.AluOpType.bypass,
                    replica_groups=[[0, 1, 2, 3], [4, 5, 6, 7]],
                    ins=[cc_in.opt()], outs=[cc_out.opt()])
                for c2 in range(NPANEL):
                    eng = nc.gpsimd if c2 % 2 == 0 else nc.sync
                    eng.dma_start(ckT[:, :, c2 * W:(c2 + 1) * W],
                                  cc_out[c2, :, 0:KB_CKV, :])
                    eng.dma_start(kpe2[:, c2 * W:(c2 + 1) * W],
                                  cc_out[c2, :, KB_CKV, :])

        with tc.tile_pool(name="qTp", bufs=1) as q_pool, \
             tc.tile_pool(name="oTp", bufs=1) as o_pool, \
             tc.tile_pool(name="phE", bufs=3) as pe_pool:
            qnopeT = q_pool.tile([P, MB_NOPE, W], F16)       # 16 KB/part
            qpeT = q_pool.tile([P, MB_PE, W], F16)           # 8 KB/part
            oT_sb = o_pool.tile([P, NH, W], F16)             # 16 KB/part
            # group 0's attention weights, prefetched ahead of phase C's
            # wqb stream so phase D starts without a DMA hiccup
            wv0 = o_pool.tile([P, KB_CKV, 2 * DV], F16)
            nc.sync.dma_start(wv0[:], t["wv_p"][:, 0, :, :])
            wkn01 = o_pool.tile([P, 2, KB_CKV, DN], F16)
            for hh in range(2):
                nc.sync.dma_start(wkn01[:, hh, :, :], t["wkn_p"][:, hh, :, :])

            # ---- phase C: qT panel (+ RoPE on pe part) ----
            # w_qb fully resident (kills the per-block stream stalls that
            # keep resetting the PE pstate ramp)
            with tc.tile_pool(name="phC_w", bufs=1) as pcw, \
                 tc.tile_pool(name="phC", bufs=2) as pc, \
                 tc.tile_pool(name="psA", bufs=2, space="PSUM") as psA:
                wqb_all = pcw.tile([P, MB_NOPE + MB_PE, KB_QLR, P], F16)
                for m in range(MB_NOPE + MB_PE):
                    eng = nc.sync if m % 2 == 0 else nc.scalar
                    eng.dma_start(wqb_all[:, m, :, :],
                                  t["w_qb_p"][:, m, :, :])
                for m in range(MB_NOPE + MB_PE):
                    ps = psA.tile([P, W], F32, tag="psA")
                    for k in range(KB_QLR):
                        _mm(nc, ps[:], wqb_all[:, m, k, :], qaT[:, k, :],
                            k == 0, k == KB_QLR - 1)
                    if m < MB_NOPE:
                        nc.scalar.activation(qnopeT[:, m, :], ps[:], COPY)
                    else:
                        j = m - MB_NOPE
                        rotq = pc.tile([P, W], F32, tag="rotq")
                        for h in (0, DR):
                            nc.vector.tensor_copy(rotq[h:h + 32, :],
                                                  ps[h + 32:h + 64, :])
                            nc.vector.tensor_copy(rotq[h + 32:h + 64, :],
                                                  ps[h:h + 32, :])
                        nc.vector.tensor_tensor(rotq[:], rotq[:],
                                                sin2sp[:], MULT)
                        tmp = pc.tile([P, W], F32, tag="tmpq")
                        nc.vector.tensor_tensor(tmp[:], ps[:],
                                                cos2p[:], MULT)
                        with nc.allow_low_precision(reason="fp16 act"):
                            nc.vector.tensor_tensor(qpeT[:, j, :], tmp[:],
                                                    rotq[:], ADD)

            # ---- phase D: per 2-head group: V, knope, attention ----
            with tc.tile_pool(name="wo", bufs=1) as wo_pool, \
                 tc.tile_pool(name="phD", bufs=2) as pd, \
                 tc.tile_pool(name="phD_v", bufs=2) as pdv, \
                 tc.tile_pool(name="phD_k", bufs=2) as pdk, \
                 tc.tile_pool(name="phD_w", bufs=2) as pdw, \
                 tc.tile_pool(name="probs", bufs=4) as pprob, \
                 tc.tile_pool(name="psSc", bufs=4, space="PSUM") as psSc, \
                 tc.tile_pool(name="psO", bufs=2, space="PSUM") as psO, \
                 ExitStack() as dctx:
                # prefetch the whole o_proj weight during attention
                # (gpsimd queue: don't head-of-line block the wv/wkn loads)
                wo_all = wo_pool.tile([P, MB_HID, NH, P], F16)  # 64 KB/part
                for m in range(MB_HID):
                    nc.gpsimd.dma_start(wo_all[:, m, :, :],
                                        t["w_o_p"][:, m, :, :])
                if with_mask:
                    mask_pool = dctx.enter_context(
                        tc.tile_pool(name="maskp", bufs=4))
                for g in range(NG):
                    # V for the 2 heads of this group: [k, 2*128 dv]
                    if g == 0:
                        wv = wv0
                    else:
                        wv = pdw.tile([P, KB_CKV, 2 * DV], F16, tag="wv")
                        nc.sync.dma_start(wv[:], t["wv_p"][:, g, :, :])
                    v_sb = pdv.tile([P, KB_S, 2 * DV], F16, tag="v")
                    for kb in range(KB_S):
                        psv = psSc.tile([P, W], F32, tag="pss")
                        for kc in range(KB_CKV):
                            _mm(nc, psv[:, :2 * DV],
                                ckT[:, kc, kb * P:(kb + 1) * P],
                                wv[:, kc, :], kc == 0, kc == KB_CKV - 1)
                        nc.scalar.activation(v_sb[:, kb, :],
                                             psv[:, :2 * DV], COPY)

                    for hl in range(2):
                        h = g * 2 + hl
                        # knopeT for head h: [128 d, S]
                        if h < 2:
                            wkn = wkn01[:, h, :, :]
                        else:
                            wkn = pdw.tile([P, KB_CKV, DN], F16, tag="wkn")
                            nc.sync.dma_start(wkn[:], t["wkn_p"][:, h, :, :])
                        knT = pdk.tile([P, NPANEL, W], F16, tag="knT")
                        for nch in range(NPANEL):
                            psk = psSc.tile([P, W], F32, tag="pss")
                            for kc in range(KB_CKV):
                                _mm(nc, psk[:], wkn[:, kc, :],
                                    ckT[:, kc, nch * W:(nch + 1) * W],
                                    kc == 0, kc == KB_CKV - 1)
                            nc.scalar.activation(knT[:, nch, :], psk[:],
                                                 COPY)

                        # attention for head h over all key blocks
                        po = psO.tile([P, W], F32, tag="po")
                        acc = pd.tile([P, W], F16, tag="acc")
                        hp64 = hl * DR
                        for kb in range(KB_S):
                            pss = psSc.tile([P, W], F32, tag="pss")
                            _mm(nc, pss[:],
                                knT[:, kb // 4, (kb % 4) * P:
                                    (kb % 4 + 1) * P],
                                qnopeT[:, h, :], True, False)
                            _mm(nc, pss[:],
                                kpe2[hp64:hp64 + DR, kb * P:(kb + 1) * P],
                                qpeT[hp64:hp64 + DR, g, :], False, True)
                            probs = pprob.tile([P, W], F16, tag="probs")
                            if with_mask:
                                mtile = mask_pool.tile([P, W], F16,
                                                       tag="mt")
                                nc.sync.dma_start(mtile[:],
                                                  t["maskT_p"][:, kb, :])
                                with nc.allow_low_precision(
                                        reason="fp16 probs"):
                                    nc.vector.scalar_tensor_tensor(
                                        probs[:], pss[:], SCALE, mtile[:],
                                        MULT, ADD)
                                nc.scalar.activation(probs[:], probs[:],
                                                     EXP)
                            else:
                                nc.scalar.activation(probs[:], pss[:],
                                                     EXP, scale=SCALE)
                            with nc.allow_low_precision(reason="fp16 acc"):
                                if kb == 0:
                                    nc.vector.tensor_copy(acc[:], probs[:])
                                else:
                                    nc.vector.tensor_tensor(
                                        acc[:], acc[:], probs[:], ADD)
                            _mm(nc, po[:],
                                v_sb[:, kb, hl * DV:(hl + 1) * DV],
                                probs[:], kb == 0, kb == KB_S - 1)
                        sums = pd.tile([P, W], F32, tag="sums")
                        nc.gpsimd.partition_all_reduce(
                            sums[:], acc[:], P, ReduceOp.add)
                        rec = pd.tile([P, W], F32, tag="rec")
                        nc.vector.reciprocal(rec[:], sums[:])
                        with nc.allow_low_precision(reason="fp16 out"):
                            nc.vector.tensor_tensor(oT_sb[:, h, :], po[:],
                                                    rec[:], MULT)

            # ------------- phase E: o_proj (all-resident) ---------------
            pe = pe_pool
            with tc.tile_pool(name="psA", bufs=2, space="PSUM") as psA:
                for m in range(MB_HID):
                    ps = psA.tile([P, W], F32, tag="psA")
                    for k in range(NH):
                        _mm(nc, ps[:], wo_all[:, m, k, :], oT_sb[:, k, :],
                            k == 0, k == NH - 1)
                    osb = pe.tile([P, W], F32, tag="osb")
                    nc.scalar.activation(osb[:], ps[:], COPY)
                    nc.sync.dma_start(t["outT"][m * P:(m + 1) * P, :], osb[:])


def _build_program(with_mask):
    nc = bacc.Bacc("TRN2", target_bir_lowering=False, debug=False,
                   num_devices=NCORES)
    t = {}

    def inp(name, shape, dt=F16):
        t[name] = nc.dram_tensor(name, list(shape), dt,
                                 kind="ExternalInput").ap()

    inp("hsT_panel_p", [P, KB_HID, W])
    inp("w_qa_p", [P, MB_QLR, KB_HID, P])
    inp("w_qb_p", [P, MB_NOPE + MB_PE, KB_QLR, P])
    inp("w_kva_p", [P, KB_HID, KVLR + DR])
    inp("wkn_p", [P, NH, KB_CKV, DN])
    inp("wv_p", [P, NG, KB_CKV, 2 * DV])
    inp("w_o_p", [P, MB_HID, NH, P])
    inp("qa_ln_p", [P, KB_QLR], F32)
    inp("kva_ln_p", [P, KB_CKV], F32)
    inp("cos2p", [P, W], F32)
    inp("sin2sp", [P, W], F32)
    if with_mask:
        inp("maskT_p", [P, KB_S, W])
    t["outT"] = nc.dram_tensor("outT", [HID, W], F32,
                               kind="ExternalOutput").ap()

    with tile.TileContext(nc) as tc:
        _emit(tc, t, with_mask)
    nc.compile()
    return nc


_PROG_CACHE = {}


def _get_program(with_mask):
    if with_mask not in _PROG_CACHE:
        _PROG_CACHE[with_mask] = _build_program(with_mask)
    return _PROG_CACHE[with_mask]


def _pkc(w, kb, mb):
    """[kb*P, mb*P] -> [P, mb, kb, P] host layout (p, m, k, c)."""
    return np.ascontiguousarray(
        np.asarray(w).reshape(kb, P, mb, P).transpose(1, 2, 0, 3)
        .astype(np.float16))


def make_in_maps(hidden_states, attention_mask, cos, sin, w_qa, qa_ln, w_qb,
                 w_kva, kva_ln, w_kvb, w_o, with_mask):
    """Host-side prep: transposes/reorders/fp16 casts; 8 input dicts."""
    f32 = np.float32
    f16 = np.float16
    c = np.ascontiguousarray

    w_qb_r = np.asarray(w_qb).reshape(QLR, NH, DQK)
    w_qb_re = np.concatenate(
        [w_qb_r[:, :, :DN].reshape(QLR, NH * DN),
         w_qb_r[:, :, DN:].reshape(QLR, NH * DR)], axis=1)
    w_kvb_r = np.asarray(w_kvb).reshape(KVLR, NH, DN + DV)
    wkn_p = c(w_kvb_r[:, :, :DN].reshape(KB_CKV, P, NH, DN)
              .transpose(1, 2, 0, 3).astype(f16))
    wv_p = c(w_kvb_r[:, :, DN:].reshape(KVLR, NG, 2 * DV)
             .reshape(KB_CKV, P, NG, 2 * DV).transpose(1, 2, 0, 3)
             .astype(f16))
    qa_ln_p = c(np.asarray(qa_ln).reshape(KB_QLR, P).T.astype(f32))
    kva_ln_p = c(np.asarray(kva_ln).reshape(KB_CKV, P).T.astype(f32))

    cosT = np.asarray(cos).T.astype(f32)                  # [64, S]
    sinT = np.asarray(sin).T.astype(f32)
    sin_s = np.concatenate([-sinT[:DR // 2], sinT[DR // 2:]], axis=0)
    cos2 = np.concatenate([cosT, cosT], axis=0)           # [128, S]
    sin2s = np.concatenate([sin_s, sin_s], axis=0)

    shared = {
        "w_qa_p": _pkc(w_qa, KB_HID, MB_QLR),
        "w_qb_p": _pkc(w_qb_re, KB_QLR, MB_NOPE + MB_PE),
        "w_kva_p": c(np.asarray(w_kva).reshape(KB_HID, P, KVLR + DR)
                     .transpose(1, 0, 2).astype(f16)),
        "wkn_p": wkn_p,
        "wv_p": wv_p,
        "w_o_p": _pkc(w_o, KB_S, MB_HID),
        "qa_ln_p": qa_ln_p,
        "kva_ln_p": kva_ln_p,
    }

    hs = np.asarray(hidden_states)
    am = np.asarray(attention_mask)
    in_maps = []
    for core in range(NCORES):
        b, pnl = divmod(core, NPANEL)
        q0 = pnl * W
        m = dict(shared)
        m["hsT_panel_p"] = c(hs[b, q0:q0 + W, :].T.reshape(KB_HID, P, W)
                             .transpose(1, 0, 2).astype(f16))
        m["cos2p"] = c(cos2[:, q0:q0 + W])
        m["sin2sp"] = c(sin2s[:, q0:q0 + W])
        if with_mask:
            mk = np.maximum(am[b, 0, q0:q0 + W, :].T, -30000.0)  # [S, W]
            m["maskT_p"] = c(mk.reshape(KB_S, P, W).transpose(1, 0, 2)
                             .astype(f16))
        in_maps.append(m)
    return in_maps


def kernel(hidden_states, attention_mask, cos, sin, w_qa, qa_ln, w_qb,
           w_kva, kva_ln, w_kvb, w_o):
    global LAST_RESULT
    with_mask = bool(np.any(np.asarray(attention_mask) != 0))
    nc = _get_program(with_mask)
    in_maps = make_in_maps(hidden_states, attention_mask, cos, sin, w_qa,
                           qa_ln, w_qb, w_kva, kva_ln, w_kvb, w_o, with_mask)
    trace = os.environ.get("KERNEL_TRACE", "0") == "1"
    res = bass_utils.run_bass_kernel_spmd(
        nc, in_maps, core_ids=list(range(NCORES)), trace=trace)
    LAST_RESULT = res

    out = np.empty((B, S, HID), np.float32)
    for core in range(NCORES):
        b, pnl = divmod(core, NPANEL)
        q0 = pnl * W
        out[b, q0:q0 + W, :] = res.results[core]["outT"].T
    return out


# revision 50
# speedup vs baseline: 1.0502x; 1.0502x over previous
"""DeepseekV2 MLA attention forward — Trainium2 Bass kernel (8 NeuronCores).

Sharding: data-parallel over batch (2) x sequence-panel-parallel over query
rows (4 panels of 512) = 8 cores. Each core computes, for its (batch, panel):
  - q path (q_a_proj -> rmsnorm -> q_b_proj) for its 512 query rows, all heads
  - kv path (kv_a_proj -> rmsnorm -> RoPE) for its OWN 512 keys, then a
    4-core AllGather over the panel group assembles the full-S compressed kv
  - kv_b expansion, full attention (16 heads) for its query rows, o_proj
Output panels are concatenated on the host.

v3 vs v2:
  - every DRAM tensor is pre-laid-out on the host so each dma_start moves
    large contiguous runs per partition (big descriptors, no rearranges)
  - rmsnorm sum-of-squares via DVE square/accumulate + GpSimd partition
    reduce: the PE stream in phases A/B is pure GEMMs (no norm bubbles)
  - kv_a computed for S/4 keys per core + AllGather (was 4x redundant)
  - PSUM->SBUF copies of V / k_nope moved from Scalar (near-critical) to DVE
Everything on-chip is fp16 except fp32 PSUM accumulation and norm/rope math.
"""

import os
import numpy as np
from contextlib import ExitStack

import concourse.bass as bass
import concourse.bacc as bacc
import concourse.mybir as mybir
import concourse.tile as tile
from concourse import bass_utils
from concourse.bass_isa import ReduceOp



B, S, HID = 2, 2048, 2048
NH = 16
QLR, KVLR = 1536, 512
DN, DR, DV = 128, 64, 128
DQK = DN + DR
SCALE = DQK ** -0.5
EPS = 1e-6
P = 128
NPANEL = 4
W = S // NPANEL            # 512 query rows per core
NCORES = B * NPANEL

F32 = mybir.dt.float32
F16 = mybir.dt.float16
EXP = mybir.ActivationFunctionType.Exp
SQRT = mybir.ActivationFunctionType.Sqrt
COPY = mybir.ActivationFunctionType.Copy
MULT = mybir.AluOpType.mult
ADD = mybir.AluOpType.add

KB_HID = HID // P          # 16
KB_QLR = QLR // P          # 12
KB_CKV = KVLR // P         # 4
KB_S = S // P              # 16
MB_QLR = QLR // P          # 12
MB_NOPE = NH * DN // P     # 16
MB_PE = NH * DR // P       # 8
MB_HID = HID // P          # 16
NG = NH // 2               # head-pair groups

LAST_RESULT = None         # BassKernelResults of the most recent launch


def _mm(nc, out, lhsT, rhs, start, stop):
    nc.tensor.matmul(out, lhsT, rhs, start=start, stop=stop)


def _emit(tc, t, with_mask):
    """Emit the whole per-core program. `t` maps tensor name -> DRAM AP."""
    nc = tc.nc

    with ExitStack() as big:
        const = big.enter_context(tc.tile_pool(name="const", bufs=1))
        qa_ln = const.tile([P, KB_QLR], F32)
        nc.sync.dma_start(qa_ln[:], t["qa_ln_p"][:])
        kva_ln = const.tile([P, KB_CKV], F32)
        nc.sync.dma_start(kva_ln[:], t["kva_ln_p"][:])
        cos2p = const.tile([P, W], F32)
        nc.sync.dma_start(cos2p[:], t["cos2p"][:])
        sin2sp = const.tile([P, W], F32)
        nc.sync.dma_start(sin2sp[:], t["sin2sp"][:])
        epsP = const.tile([P, 1], F32)
        nc.vector.memset(epsP[:], EPS)

        # resident activations (allocated before the A/B pool; LIFO free)
        qa_pool = big.enter_context(tc.tile_pool(name="qaT", bufs=1))
        qaT = qa_pool.tile([P, KB_QLR, W], F16)          # 12 KB/part
        ckv_pool = big.enter_context(tc.tile_pool(name="ckv", bufs=1))
        ckT = ckv_pool.tile([P, KB_CKV, S], F16)         # 16 KB/part
        kpe2 = ckv_pool.tile([P, S], F16)                # 4 KB/part
        dram = big.enter_context(tc.tile_pool(name="ccd", bufs=1,
                                              space="DRAM"))
        cc_in = dram.tile([P, KB_CKV + 1, W], F16)
        cc_out = dram.tile([NPANEL, P, KB_CKV + 1, W], F16)

        def rms_inv(pool, sqacc, inv_dim):
            """[P,W] broadcast 1/sqrt(mean+eps) from f16 partial squares."""
            sums = pool.tile([P, W], F32, tag="sums")
            nc.gpsimd.partition_all_reduce(sums[:], sqacc[:], P, ReduceOp.add)
            srt = pool.tile([P, W], F32, tag="srt")
            nc.scalar.activation(srt[:], sums[:], SQRT, bias=epsP[:],
                                 scale=inv_dim)
            rq = pool.tile([P, W], F32, tag="rq")
            nc.vector.reciprocal(rq[:], srt[:])
            return rq

        # ---------------- phases A+B (panel chunk of hidden) ---------------
        with tc.tile_pool(name="ab", bufs=1) as ab_pool:
            hp = ab_pool.tile([P, KB_HID, W], F16)       # 16 KB/part
            nc.sync.dma_start(hp[:], t["hsT_panel_p"][:])
            wkva = ab_pool.tile([P, KB_HID, KVLR + DR], F16)  # 18 KB/part
            # kpe columns first: phase B's m-loop starts with the kpe block
            nc.gpsimd.dma_start(wkva[:, :, KB_CKV * P:],
                                t["w_kva_p"][:, :, KB_CKV * P:])
            nc.gpsimd.dma_start(wkva[:, :, :KB_CKV * P],
                                t["w_kva_p"][:, :, :KB_CKV * P])
            # w_qa fully resident: its region must not alias phase B's
            # staging pools, and the loads overlap phase B's GEMMs
            wqa_all = ab_pool.tile([P, MB_QLR, KB_HID, P], F16)  # 48 KB/part
            for m in range(MB_QLR):
                nc.sync.dma_start(wqa_all[:, m, :, :], t["w_qa_p"][:, m, :, :])

            # ---- phases B then A in ONE scope: a single uninterrupted
            # PE stream of 17 GEMM chains; B first so the kv AllGather
            # launches early and overlaps the rest of A and phase C ----
            with tc.tile_pool(name="phAB", bufs=2) as pab, \
                 tc.tile_pool(name="phB_c", bufs=1) as pbc, \
                 tc.tile_pool(name="psA", bufs=3, space="PSUM") as psA:
                stg = pbc.tile([P, KB_CKV + 1, W], F16)  # 5 KB/part staging
                kpf = pbc.tile([P, W], F32)
                # ---- phase A chains (same scope, PE never goes cold) ----
                sqaccA = pab.tile([P, W], F16, tag="sqaccA")
                for m in range(MB_QLR):
                    ps = psA.tile([P, W], F32, tag="psA")
                    for k in range(KB_HID):
                        _mm(nc, ps[:], wqa_all[:, m, k, :], hp[:, k, :],
                            k == 0, k == KB_HID - 1)
                    nc.scalar.activation(qaT[:, m, :], ps[:], COPY)
                    sq = pab.tile([P, W], F16, tag="sq")
                    with nc.allow_low_precision(reason="fp16 sumsq"):
                        nc.vector.tensor_tensor(sq[:], qaT[:, m, :],
                                                qaT[:, m, :], MULT)
                        if m == 0:
                            nc.vector.tensor_copy(sqaccA[:], sq[:])
                        else:
                            nc.vector.tensor_tensor(sqaccA[:], sqaccA[:],
                                                    sq[:], ADD)
                rq = rms_inv(pab, sqaccA, 1.0 / QLR)
                for m in range(MB_QLR):
                    with nc.allow_low_precision(reason="fp16 activations"):
                        nc.vector.scalar_tensor_tensor(
                            qaT[:, m, :], qaT[:, m, :], qa_ln[:, m:m + 1],
                            rq[:], MULT, MULT)

                sqaccB = pab.tile([P, W], F16, tag="sqaccB")
                for m in (KB_CKV, 0, 1, 2, 3):   # kpe first: rope starts early
                    rows = P if m < KB_CKV else DR
                    ps = psA.tile([P, W], F32, tag="psA")
                    for k in range(KB_HID):
                        _mm(nc, ps[:rows, :], wkva[:, k, m * P:m * P + rows],
                            hp[:, k, :], k == 0, k == KB_HID - 1)
                    if m < KB_CKV:
                        nc.scalar.activation(stg[:, m, :], ps[:], COPY)
                        sq = pab.tile([P, W], F16, tag="sq")
                        with nc.allow_low_precision(reason="fp16 sumsq"):
                            nc.vector.tensor_tensor(sq[:], stg[:, m, :],
                                                    stg[:, m, :], MULT)
                            if m == 0:
                                nc.vector.tensor_copy(sqaccB[:], sq[:])
                            else:
                                nc.vector.tensor_tensor(sqaccB[:], sqaccB[:],
                                                        sq[:], ADD)
                    else:
                        nc.scalar.activation(kpf[0:DR, :], ps[0:DR, :], COPY)
                        nc.vector.tensor_copy(kpf[DR:P, :], ps[0:DR, :])
                        # rope immediately (overlaps the ckv GEMMs)
                        rot = pbc.tile([P, W], F32, tag="rot")
                        for hh in (0, DR):
                            nc.vector.tensor_copy(rot[hh:hh + 32, :],
                                                  kpf[hh + 32:hh + 64, :])
                            nc.vector.tensor_copy(rot[hh + 32:hh + 64, :],
                                                  kpf[hh:hh + 32, :])
                        nc.vector.tensor_tensor(kpf[:], kpf[:], cos2p[:],
                                                MULT)
                        nc.vector.tensor_tensor(rot[:], rot[:], sin2sp[:],
                                                MULT)
                        with nc.allow_low_precision(reason="fp16 act"):
                            nc.vector.tensor_tensor(stg[:, KB_CKV, :],
                                                    kpf[:], rot[:], ADD)
                rk = rms_inv(pab, sqaccB, 1.0 / KVLR)
                for m in range(KB_CKV):
                    with nc.allow_low_precision(reason="fp16 activations"):
                        nc.vector.scalar_tensor_tensor(
                            stg[:, m, :], stg[:, m, :],
                            kva_ln[:, m:m + 1], rk[:], MULT, MULT)
                # all-gather the compressed kv across the 4 panel cores
                # (gpsimd issue queue: keeps the collective's semaphore
                # waits off the sync engine so phase C DMAs aren't blocked)
                nc.gpsimd.dma_start(cc_in[:], stg[:])
                nc.gpsimd.collective_compute(
                    "AllGather", mybir.AluOpType.bypass,
                    replica_groups=[[0, 1, 2, 3], [4, 5, 6, 7]],
                    ins=[cc_in.opt()], outs=[cc_out.opt()])
                for c2 in range(NPANEL):
                    nc.gpsimd.dma_start(ckT[:, :, c2 * W:(c2 + 1) * W],
                                        cc_out[c2, :, 0:KB_CKV, :])
                    nc.gpsimd.dma_start(kpe2[:, c2 * W:(c2 + 1) * W],
                                        cc_out[c2, :, KB_CKV, :])

        with tc.tile_pool(name="qTp", bufs=1) as q_pool, \
             tc.tile_pool(name="oTp", bufs=1) as o_pool, \
             tc.tile_pool(name="phE", bufs=3) as pe_pool:
            qnopeT = q_pool.tile([P, MB_NOPE, W], F16)       # 16 KB/part
            qpeT = q_pool.tile([P, MB_PE, W], F16)           # 8 KB/part
            oT_sb = o_pool.tile([P, NH, W], F16)             # 16 KB/part
            # group 0's attention weights, prefetched ahead of phase C's
            # wqb stream so phase D starts without a DMA hiccup
            wv0 = o_pool.tile([P, KB_CKV, 2 * DV], F16)
            nc.sync.dma_start(wv0[:], t["wv_p"][:, 0, :, :])
            wkn01 = o_pool.tile([P, 2, KB_CKV, DN], F16)
            for hh in range(2):
                nc.sync.dma_start(wkn01[:, hh, :, :], t["wkn_p"][:, hh, :, :])

            # ---- phase C: qT panel (+ RoPE on pe part) ----
            # w_qb fully resident (kills the per-block stream stalls that
            # keep resetting the PE pstate ramp)
            with tc.tile_pool(name="phC_w", bufs=1) as pcw, \
                 tc.tile_pool(name="phC", bufs=2) as pc, \
                 tc.tile_pool(name="psA", bufs=2, space="PSUM") as psA:
                wqb_all = pcw.tile([P, MB_NOPE + MB_PE, KB_QLR, P], F16)
                for m in range(MB_NOPE + MB_PE):
                    nc.sync.dma_start(wqb_all[:, m, :, :],
                                      t["w_qb_p"][:, m, :, :])
                for m in range(MB_NOPE + MB_PE):
                    ps = psA.tile([P, W], F32, tag="psA")
                    for k in range(KB_QLR):
                        _mm(nc, ps[:], wqb_all[:, m, k, :], qaT[:, k, :],
                            k == 0, k == KB_QLR - 1)
                    if m < MB_NOPE:
                        nc.scalar.activation(qnopeT[:, m, :], ps[:], COPY)
                    else:
                        j = m - MB_NOPE
                        rotq = pc.tile([P, W], F32, tag="rotq")
                        for h in (0, DR):
                            nc.vector.tensor_copy(rotq[h:h + 32, :],
                                                  ps[h + 32:h + 64, :])
                            nc.vector.tensor_copy(rotq[h + 32:h + 64, :],
                                                  ps[h:h + 32, :])
                        nc.vector.tensor_tensor(rotq[:], rotq[:],
                                                sin2sp[:], MULT)
                        tmp = pc.tile([P, W], F32, tag="tmpq")
                        nc.vector.tensor_tensor(tmp[:], ps[:],
                                                cos2p[:], MULT)
                        with nc.allow_low_precision(reason="fp16 act"):
                            nc.vector.tensor_tensor(qpeT[:, j, :], tmp[:],
                                                    rotq[:], ADD)

            # ---- phase D: per 2-head group: V, knope, attention ----
            with tc.tile_pool(name="wo", bufs=1) as wo_pool, \
                 tc.tile_pool(name="phD", bufs=2) as pd, \
                 tc.tile_pool(name="phD_v", bufs=2) as pdv, \
                 tc.tile_pool(name="phD_k", bufs=2) as pdk, \
                 tc.tile_pool(name="phD_w", bufs=2) as pdw, \
                 tc.tile_pool(name="probs", bufs=4) as pprob, \
                 tc.tile_pool(name="psSc", bufs=4, space="PSUM") as psSc, \
                 tc.tile_pool(name="psO", bufs=2, space="PSUM") as psO, \
                 ExitStack() as dctx:
                # prefetch the whole o_proj weight during attention
                # (gpsimd queue: don't head-of-line block the wv/wkn loads)
                wo_all = wo_pool.tile([P, MB_HID, NH, P], F16)  # 64 KB/part
                for m in range(MB_HID):
                    nc.gpsimd.dma_start(wo_all[:, m, :, :],
                                        t["w_o_p"][:, m, :, :])
                if with_mask:
                    mask_pool = dctx.enter_context(
                        tc.tile_pool(name="maskp", bufs=4))
                for g in range(NG):
                    # V for the 2 heads of this group: [k, 2*128 dv]
                    if g == 0:
                        wv = wv0
                    else:
                        wv = pdw.tile([P, KB_CKV, 2 * DV], F16, tag="wv")
                        nc.sync.dma_start(wv[:], t["wv_p"][:, g, :, :])
                    v_sb = pdv.tile([P, KB_S, 2 * DV], F16, tag="v")
                    for kb in range(KB_S):
                        psv = psSc.tile([P, W], F32, tag="pss")
                        for kc in range(KB_CKV):
                            _mm(nc, psv[:, :2 * DV],
                                ckT[:, kc, kb * P:(kb + 1) * P],
                                wv[:, kc, :], kc == 0, kc == KB_CKV - 1)
                        nc.scalar.activation(v_sb[:, kb, :],
                                             psv[:, :2 * DV], COPY)

                    for hl in range(2):
                        h = g * 2 + hl
                        # knopeT for head h: [128 d, S]
                        if h < 2:
                            wkn = wkn01[:, h, :, :]
                        else:
                            wkn = pdw.tile([P, KB_CKV, DN], F16, tag="wkn")
                            nc.sync.dma_start(wkn[:], t["wkn_p"][:, h, :, :])
                        knT = pdk.tile([P, NPANEL, W], F16, tag="knT")
                        for nch in range(NPANEL):
                            psk = psSc.tile([P, W], F32, tag="pss")
                            for kc in range(KB_CKV):
                                _mm(nc, psk[:], wkn[:, kc, :],
                                    ckT[:, kc, nch * W:(nch + 1) * W],
                                    kc == 0, kc == KB_CKV - 1)
                            nc.scalar.activation(knT[:, nch, :], psk[:],
                                                 COPY)

                        # attention for head h over all key blocks
                        po = psO.tile([P, W], F32, tag="po")
                        acc = pd.tile([P, W], F16, tag="acc")
                        hp64 = hl * DR
                        for kb in range(KB_S):
                            pss = psSc.tile([P, W], F32, tag="pss")
                            _mm(nc, pss[:],
                                knT[:, kb // 4, (kb % 4) * P:
                                    (kb % 4 + 1) * P],
                                qnopeT[:, h, :], True, False)
                            _mm(nc, pss[:],
                                kpe2[hp64:hp64 + DR, kb * P:(kb + 1) * P],
                                qpeT[hp64:hp64 + DR, g, :], False, True)
                            probs = pprob.tile([P, W], F16, tag="probs")
                            if with_mask:
                                mtile = mask_pool.tile([P, W], F16,
                                                       tag="mt")
                                nc.sync.dma_start(mtile[:],
                                                  t["maskT_p"][:, kb, :])
                                with nc.allow_low_precision(
                                        reason="fp16 probs"):
                                    nc.vector.scalar_tensor_tensor(
                                        probs[:], pss[:], SCALE, mtile[:],
                                        MULT, ADD)
                                nc.scalar.activation(probs[:], probs[:],
                                                     EXP)
                            else:
                                nc.scalar.activation(probs[:], pss[:],
                                                     EXP, scale=SCALE)
                            with nc.allow_low_precision(reason="fp16 acc"):
                                if kb == 0:
                                    nc.vector.tensor_copy(acc[:], probs[:])
                                else:
                                    nc.vector.tensor_tensor(
                                        acc[:], acc[:], probs[:], ADD)
                            _mm(nc, po[:],
                                v_sb[:, kb, hl * DV:(hl + 1) * DV],
                                probs[:], kb == 0, kb == KB_S - 1)
                        sums = pd.tile([P, W], F32, tag="sums")
                        nc.gpsimd.partition_all_reduce(
                            sums[:], acc[:], P, ReduceOp.add)
                        rec = pd.tile([P, W], F32, tag="rec")
                        nc.vector.reciprocal(rec[:], sums[:])
                        with nc.allow_low_precision(reason="fp16 out"):
                            nc.vector.tensor_tensor(oT_sb[:, h, :], po[:],
                                                    rec[:], MULT)

            # ------------- phase E: o_proj (all-resident) ---------------
            pe = pe_pool
            with tc.tile_pool(name="psA", bufs=2, space="PSUM") as psA:
                for m in range(MB_HID):
                    ps = psA.tile([P, W], F32, tag="psA")
                    for k in range(NH):
                        _mm(nc, ps[:], wo_all[:, m, k, :], oT_sb[:, k, :],
                            k == 0, k == NH - 1)
                    osb = pe.tile([P, W], F32, tag="osb")
                    nc.scalar.activation(osb[:], ps[:], COPY)
                    nc.sync.dma_start(t["outT"][m * P:(m + 1) * P, :], osb[:])


def _build_program(with_mask):
    nc = bacc.Bacc("TRN2", target_bir_lowering=False, debug=False,
                   num_devices=NCORES)
    t = {}

    def inp(name, shape, dt=F16):
        t[name] = nc.dram_tensor(name, list(shape), dt,
                                 kind="ExternalInput").ap()

    inp("hsT_panel_p", [P, KB_HID, W])
    inp("w_qa_p", [P, MB_QLR, KB_HID, P])
    inp("w_qb_p", [P, MB_NOPE + MB_PE, KB_QLR, P])
    inp("w_kva_p", [P, KB_HID, KVLR + DR])
    inp("wkn_p", [P, NH, KB_CKV, DN])
    inp("wv_p", [P, NG, KB_CKV, 2 * DV])
    inp("w_o_p", [P, MB_HID, NH, P])
    inp("qa_ln_p", [P, KB_QLR], F32)
    inp("kva_ln_p", [P, KB_CKV], F32)
    inp("cos2p", [P, W], F32)
    inp("sin2sp", [P, W], F32)
    if with_mask:
        inp("maskT_p", [P, KB_S, W])
    t["outT"] = nc.dram_tensor("outT", [HID, W], F32,
                               kind="ExternalOutput").ap()

    with tile.TileContext(nc) as tc:
        _emit(tc, t, with_mask)
    nc.compile()
    return nc


_PROG_CACHE = {}


def _get_program(with_mask):
    if with_mask not in _PROG_CACHE:
        _PROG_CACHE[with_mask] = _build_program(with_mask)
    return _PROG_CACHE[with_mask]


def _pkc(w, kb, mb):
    """[kb*P, mb*P] -> [P, mb, kb, P] host layout (p, m, k, c)."""
    return np.ascontiguousarray(
        np.asarray(w).reshape(kb, P, mb, P).transpose(1, 2, 0, 3)
        .astype(np.float16))


def make_in_maps(hidden_states, attention_mask, cos, sin, w_qa, qa_ln, w_qb,
                 w_kva, kva_ln, w_kvb, w_o, with_mask):
    """Host-side prep: transposes/reorders/fp16 casts; 8 input dicts."""
    f32 = np.float32
    f16 = np.float16
    c = np.ascontiguousarray

    w_qb_r = np.asarray(w_qb).reshape(QLR, NH, DQK)
    w_qb_re = np.concatenate(
        [w_qb_r[:, :, :DN].reshape(QLR, NH * DN),
         w_qb_r[:, :, DN:].reshape(QLR, NH * DR)], axis=1)
    w_kvb_r = np.asarray(w_kvb).reshape(KVLR, NH, DN + DV)
    wkn_p = c(w_kvb_r[:, :, :DN].reshape(KB_CKV, P, NH, DN)
              .transpose(1, 2, 0, 3).astype(f16))
    wv_p = c(w_kvb_r[:, :, DN:].reshape(KVLR, NG, 2 * DV)
             .reshape(KB_CKV, P, NG, 2 * DV).transpose(1, 2, 0, 3)
             .astype(f16))
    qa_ln_p = c(np.asarray(qa_ln).reshape(KB_QLR, P).T.astype(f32))
    kva_ln_p = c(np.asarray(kva_ln).reshape(KB_CKV, P).T.astype(f32))

    cosT = np.asarray(cos).T.astype(f32)                  # [64, S]
    sinT = np.asarray(sin).T.astype(f32)
    sin_s = np.concatenate([-sinT[:DR // 2], sinT[DR // 2:]], axis=0)
    cos2 = np.concatenate([cosT, cosT], axis=0)           # [128, S]
    sin2s = np.concatenate([sin_s, sin_s], axis=0)

    shared = {
        "w_qa_p": _pkc(w_qa, KB_HID, MB_QLR),
        "w_qb_p": _pkc(w_qb_re, KB_QLR, MB_NOPE + MB_PE),
        "w_kva_p": c(np.asarray(w_kva).reshape(KB_HID, P, KVLR + DR)
                     .transpose(1, 0, 2).astype(f16)),
        "wkn_p": wkn_p,
        "wv_p": wv_p,
        "w_o_p": _pkc(w_o, KB_S, MB_HID),
        "qa_ln_p": qa_ln_p,
        "kva_ln_p": kva_ln_p,
    }

    hs = np.asarray(hidden_states)
    am = np.asarray(attention_mask)
    in_maps = []
    for core in range(NCORES):
        b, pnl = divmod(core, NPANEL)
        q0 = pnl * W
        m = dict(shared)
        m["hsT_panel_p"] = c(hs[b, q0:q0 + W, :].T.reshape(KB_HID, P, W)
                             .transpose(1, 0, 2).astype(f16))
        m["cos2p"] = c(cos2[:, q0:q0 + W])
        m["sin2sp"] = c(sin2s[:, q0:q0 + W])
        if with_mask:
            mk = np.maximum(am[b, 0, q0:q0 + W, :].T, -30000.0)  # [S, W]
            m["maskT_p"] = c(mk.reshape(KB_S, P, W).transpose(1, 0, 2)
                             .astype(f16))
        in_maps.append(m)
    return in_maps


def kernel(hidden_states, attention_mask, cos, sin, w_qa, qa_ln, w_qb,
           w_kva, kva_ln, w_kvb, w_o):
    global LAST_RESULT
    with_mask = bool(np.any(np.asarray(attention_mask) != 0))
    nc = _get_program(with_mask)
    in_maps = make_in_maps(hidden_states, attention_mask, cos, sin, w_qa,
                           qa_ln, w_qb, w_kva, kva_ln, w_kvb, w_o, with_mask)
    trace = os.environ.get("KERNEL_TRACE", "0") == "1"
    res = bass_utils.run_bass_kernel_spmd(
        nc, in_maps, core_ids=list(range(NCORES)), trace=trace)
    LAST_RESULT = res

    out = np.empty((B, S, HID), np.float32)
    for core in range(NCORES):
        b, pnl = divmod(core, NPANEL)
        q0 = pnl * W
        out[b, q0:q0 + W, :] = res.results[core]["outT"].T
    return out


# revision 51
# speedup vs baseline: 1.0800x; 1.0283x over previous
"""DeepseekV2 MLA attention forward — Trainium2 Bass kernel (8 NeuronCores).

Sharding: data-parallel over batch (2) x sequence-panel-parallel over query
rows (4 panels of 512) = 8 cores. Each core computes, for its (batch, panel):
  - q path (q_a_proj -> rmsnorm -> q_b_proj) for its 512 query rows, all heads
  - kv path (kv_a_proj -> rmsnorm -> RoPE) for its OWN 512 keys, then a
    4-core AllGather over the panel group assembles the full-S compressed kv
  - kv_b expansion, full attention (16 heads) for its query rows, o_proj
Output panels are concatenated on the host.

v3 vs v2:
  - every DRAM tensor is pre-laid-out on the host so each dma_start moves
    large contiguous runs per partition (big descriptors, no rearranges)
  - rmsnorm sum-of-squares via DVE square/accumulate + GpSimd partition
    reduce: the PE stream in phases A/B is pure GEMMs (no norm bubbles)
  - kv_a computed for S/4 keys per core + AllGather (was 4x redundant)
  - PSUM->SBUF copies of V / k_nope moved from Scalar (near-critical) to DVE
Everything on-chip is fp16 except fp32 PSUM accumulation and norm/rope math.
"""

import os
import numpy as np
from contextlib import ExitStack

import concourse.bass as bass
import concourse.bacc as bacc
import concourse.mybir as mybir
import concourse.tile as tile
from concourse import bass_utils
from concourse.bass_isa import ReduceOp



B, S, HID = 2, 2048, 2048
NH = 16
QLR, KVLR = 1536, 512
DN, DR, DV = 128, 64, 128
DQK = DN + DR
SCALE = DQK ** -0.5
EPS = 1e-6
P = 128
NPANEL = 4
W = S // NPANEL            # 512 query rows per core
NCORES = B * NPANEL

F32 = mybir.dt.float32
F16 = mybir.dt.float16
EXP = mybir.ActivationFunctionType.Exp
SQRT = mybir.ActivationFunctionType.Sqrt
COPY = mybir.ActivationFunctionType.Copy
MULT = mybir.AluOpType.mult
ADD = mybir.AluOpType.add

KB_HID = HID // P          # 16
KB_QLR = QLR // P          # 12
KB_CKV = KVLR // P         # 4
KB_S = S // P              # 16
MB_QLR = QLR // P          # 12
MB_NOPE = NH * DN // P     # 16
MB_PE = NH * DR // P       # 8
MB_HID = HID // P          # 16
NG = NH // 2               # head-pair groups

LAST_RESULT = None         # BassKernelResults of the most recent launch


def _mm(nc, out, lhsT, rhs, start, stop):
    nc.tensor.matmul(out, lhsT, rhs, start=start, stop=stop)


def _emit(tc, t, with_mask):
    """Emit the whole per-core program. `t` maps tensor name -> DRAM AP."""
    nc = tc.nc

    with ExitStack() as big:
        const = big.enter_context(tc.tile_pool(name="const", bufs=1))
        qa_ln = const.tile([P, KB_QLR], F32)
        nc.sync.dma_start(qa_ln[:], t["qa_ln_p"][:])
        kva_ln = const.tile([P, KB_CKV], F32)
        nc.sync.dma_start(kva_ln[:], t["kva_ln_p"][:])
        cos2p = const.tile([P, W], F32)
        nc.sync.dma_start(cos2p[:], t["cos2p"][:])
        sin2sp = const.tile([P, W], F32)
        nc.sync.dma_start(sin2sp[:], t["sin2sp"][:])
        epsP = const.tile([P, 1], F32)
        nc.vector.memset(epsP[:], EPS)

        # resident activations (allocated before the A/B pool; LIFO free)
        qa_pool = big.enter_context(tc.tile_pool(name="qaT", bufs=1))
        qaT = qa_pool.tile([P, KB_QLR, W], F16)          # 12 KB/part
        ckv_pool = big.enter_context(tc.tile_pool(name="ckv", bufs=1))
        ckT = ckv_pool.tile([P, KB_CKV, S], F16)         # 16 KB/part
        kpe2 = ckv_pool.tile([P, S], F16)                # 4 KB/part
        dram = big.enter_context(tc.tile_pool(name="ccd", bufs=1,
                                              space="DRAM"))
        cc_in = dram.tile([P, KB_CKV + 1, W], F16)
        cc_out = dram.tile([NPANEL, P, KB_CKV + 1, W], F16)

        def rms_inv(pool, sqacc, inv_dim):
            """[P,W] broadcast 1/sqrt(mean+eps) from f16 partial squares."""
            sums = pool.tile([P, W], F32, tag="sums")
            nc.gpsimd.partition_all_reduce(sums[:], sqacc[:], P, ReduceOp.add)
            srt = pool.tile([P, W], F32, tag="srt")
            nc.scalar.activation(srt[:], sums[:], SQRT, bias=epsP[:],
                                 scale=inv_dim)
            rq = pool.tile([P, W], F32, tag="rq")
            nc.vector.reciprocal(rq[:], srt[:])
            return rq

        # ---------------- phases A+B (panel chunk of hidden) ---------------
        with tc.tile_pool(name="ab", bufs=1) as ab_pool:
            hp = ab_pool.tile([P, KB_HID, W], F16)       # 16 KB/part
            nc.sync.dma_start(hp[:], t["hsT_panel_p"][:])
            wkva = ab_pool.tile([P, KB_HID, KVLR + DR], F16)  # 18 KB/part
            # w_qa resident, loads split across the sync and gpsimd DMA
            # rings (sync alone saturates delivering A+C weights serially);
            # wkva after the odd blocks — phase B runs after phase A
            wqa_all = ab_pool.tile([P, MB_QLR, KB_HID, P], F16)  # 48 KB/part
            for m in range(MB_QLR):
                eng = nc.sync if m % 2 == 0 else nc.gpsimd
                eng.dma_start(wqa_all[:, m, :, :], t["w_qa_p"][:, m, :, :])
            # kpe columns first: phase B's m-loop starts with the kpe block
            nc.gpsimd.dma_start(wkva[:, :, KB_CKV * P:],
                                t["w_kva_p"][:, :, KB_CKV * P:])
            nc.gpsimd.dma_start(wkva[:, :, :KB_CKV * P],
                                t["w_kva_p"][:, :, :KB_CKV * P])

            # ---- phases B then A in ONE scope: a single uninterrupted
            # PE stream of 17 GEMM chains; B first so the kv AllGather
            # launches early and overlaps the rest of A and phase C ----
            with tc.tile_pool(name="phAB", bufs=2) as pab, \
                 tc.tile_pool(name="phB_c", bufs=1) as pbc, \
                 tc.tile_pool(name="psA", bufs=3, space="PSUM") as psA:
                stg = pbc.tile([P, KB_CKV + 1, W], F16)  # 5 KB/part staging
                kpf = pbc.tile([P, W], F32)
                # ---- phase A chains (same scope, PE never goes cold) ----
                sqaccA = pab.tile([P, W], F16, tag="sqaccA")
                for m in range(MB_QLR):
                    ps = psA.tile([P, W], F32, tag="psA")
                    for k in range(KB_HID):
                        _mm(nc, ps[:], wqa_all[:, m, k, :], hp[:, k, :],
                            k == 0, k == KB_HID - 1)
                    nc.scalar.activation(qaT[:, m, :], ps[:], COPY)
                    sq = pab.tile([P, W], F16, tag="sq")
                    with nc.allow_low_precision(reason="fp16 sumsq"):
                        nc.vector.tensor_tensor(sq[:], qaT[:, m, :],
                                                qaT[:, m, :], MULT)
                        if m == 0:
                            nc.vector.tensor_copy(sqaccA[:], sq[:])
                        else:
                            nc.vector.tensor_tensor(sqaccA[:], sqaccA[:],
                                                    sq[:], ADD)
                rq = rms_inv(pab, sqaccA, 1.0 / QLR)
                for m in range(MB_QLR):
                    with nc.allow_low_precision(reason="fp16 activations"):
                        nc.vector.scalar_tensor_tensor(
                            qaT[:, m, :], qaT[:, m, :], qa_ln[:, m:m + 1],
                            rq[:], MULT, MULT)

                sqaccB = pab.tile([P, W], F16, tag="sqaccB")
                for m in (KB_CKV, 0, 1, 2, 3):   # kpe first: rope starts early
                    rows = P if m < KB_CKV else DR
                    ps = psA.tile([P, W], F32, tag="psA")
                    for k in range(KB_HID):
                        _mm(nc, ps[:rows, :], wkva[:, k, m * P:m * P + rows],
                            hp[:, k, :], k == 0, k == KB_HID - 1)
                    if m < KB_CKV:
                        nc.scalar.activation(stg[:, m, :], ps[:], COPY)
                        sq = pab.tile([P, W], F16, tag="sq")
                        with nc.allow_low_precision(reason="fp16 sumsq"):
                            nc.vector.tensor_tensor(sq[:], stg[:, m, :],
                                                    stg[:, m, :], MULT)
                            if m == 0:
                                nc.vector.tensor_copy(sqaccB[:], sq[:])
                            else:
                                nc.vector.tensor_tensor(sqaccB[:], sqaccB[:],
                                                        sq[:], ADD)
                    else:
                        nc.scalar.activation(kpf[0:DR, :], ps[0:DR, :], COPY)
                        nc.vector.tensor_copy(kpf[DR:P, :], ps[0:DR, :])
                        # rope immediately (overlaps the ckv GEMMs)
                        rot = pbc.tile([P, W], F32, tag="rot")
                        for hh in (0, DR):
                            nc.vector.tensor_copy(rot[hh:hh + 32, :],
                                                  kpf[hh + 32:hh + 64, :])
                            nc.vector.tensor_copy(rot[hh + 32:hh + 64, :],
                                                  kpf[hh:hh + 32, :])
                        nc.vector.tensor_tensor(kpf[:], kpf[:], cos2p[:],
                                                MULT)
                        nc.vector.tensor_tensor(rot[:], rot[:], sin2sp[:],
                                                MULT)
                        with nc.allow_low_precision(reason="fp16 act"):
                            nc.vector.tensor_tensor(stg[:, KB_CKV, :],
                                                    kpf[:], rot[:], ADD)
                rk = rms_inv(pab, sqaccB, 1.0 / KVLR)
                for m in range(KB_CKV):
                    with nc.allow_low_precision(reason="fp16 activations"):
                        nc.vector.scalar_tensor_tensor(
                            stg[:, m, :], stg[:, m, :],
                            kva_ln[:, m:m + 1], rk[:], MULT, MULT)
                # all-gather the compressed kv across the 4 panel cores
                # (gpsimd issue queue: keeps the collective's semaphore
                # waits off the sync engine so phase C DMAs aren't blocked)
                nc.gpsimd.dma_start(cc_in[:], stg[:])
                nc.gpsimd.collective_compute(
                    "AllGather", mybir.AluOpType.bypass,
                    replica_groups=[[0, 1, 2, 3], [4, 5, 6, 7]],
                    ins=[cc_in.opt()], outs=[cc_out.opt()])
                for c2 in range(NPANEL):
                    nc.gpsimd.dma_start(ckT[:, :, c2 * W:(c2 + 1) * W],
                                        cc_out[c2, :, 0:KB_CKV, :])
                    nc.gpsimd.dma_start(kpe2[:, c2 * W:(c2 + 1) * W],
                                        cc_out[c2, :, KB_CKV, :])

        with tc.tile_pool(name="qTp", bufs=1) as q_pool, \
             tc.tile_pool(name="oTp", bufs=1) as o_pool, \
             tc.tile_pool(name="phE", bufs=3) as pe_pool:
            qnopeT = q_pool.tile([P, MB_NOPE, W], F16)       # 16 KB/part
            qpeT = q_pool.tile([P, MB_PE, W], F16)           # 8 KB/part
            oT_sb = o_pool.tile([P, NH, W], F16)             # 16 KB/part
            # group 0's attention weights, prefetched ahead of phase C's
            # wqb stream so phase D starts without a DMA hiccup
            wv0 = o_pool.tile([P, KB_CKV, 2 * DV], F16)
            nc.sync.dma_start(wv0[:], t["wv_p"][:, 0, :, :])
            wkn01 = o_pool.tile([P, 2, KB_CKV, DN], F16)
            for hh in range(2):
                nc.sync.dma_start(wkn01[:, hh, :, :], t["wkn_p"][:, hh, :, :])

            # ---- phase C: qT panel (+ RoPE on pe part) ----
            # w_qb fully resident (kills the per-block stream stalls that
            # keep resetting the PE pstate ramp)
            with tc.tile_pool(name="phC_w", bufs=1) as pcw, \
                 tc.tile_pool(name="phC", bufs=2) as pc, \
                 tc.tile_pool(name="psA", bufs=2, space="PSUM") as psA:
                wqb_all = pcw.tile([P, MB_NOPE + MB_PE, KB_QLR, P], F16)
                for m in range(MB_NOPE + MB_PE):
                    nc.sync.dma_start(wqb_all[:, m, :, :],
                                      t["w_qb_p"][:, m, :, :])
                for m in range(MB_NOPE + MB_PE):
                    ps = psA.tile([P, W], F32, tag="psA")
                    for k in range(KB_QLR):
                        _mm(nc, ps[:], wqb_all[:, m, k, :], qaT[:, k, :],
                            k == 0, k == KB_QLR - 1)
                    if m < MB_NOPE:
                        nc.scalar.activation(qnopeT[:, m, :], ps[:], COPY)
                    else:
                        j = m - MB_NOPE
                        rotq = pc.tile([P, W], F32, tag="rotq")
                        for h in (0, DR):
                            nc.vector.tensor_copy(rotq[h:h + 32, :],
                                                  ps[h + 32:h + 64, :])
                            nc.vector.tensor_copy(rotq[h + 32:h + 64, :],
                                                  ps[h:h + 32, :])
                        nc.vector.tensor_tensor(rotq[:], rotq[:],
                                                sin2sp[:], MULT)
                        tmp = pc.tile([P, W], F32, tag="tmpq")
                        nc.vector.tensor_tensor(tmp[:], ps[:],
                                                cos2p[:], MULT)
                        with nc.allow_low_precision(reason="fp16 act"):
                            nc.vector.tensor_tensor(qpeT[:, j, :], tmp[:],
                                                    rotq[:], ADD)

            # ---- phase D: per 2-head group: V, knope, attention ----
            with tc.tile_pool(name="wo", bufs=1) as wo_pool, \
                 tc.tile_pool(name="phD", bufs=2) as pd, \
                 tc.tile_pool(name="phD_v", bufs=2) as pdv, \
                 tc.tile_pool(name="phD_k", bufs=2) as pdk, \
                 tc.tile_pool(name="phD_w", bufs=2) as pdw, \
                 tc.tile_pool(name="probs", bufs=4) as pprob, \
                 tc.tile_pool(name="psSc", bufs=4, space="PSUM") as psSc, \
                 tc.tile_pool(name="psO", bufs=2, space="PSUM") as psO, \
                 ExitStack() as dctx:
                # prefetch the whole o_proj weight during attention
                # (gpsimd queue: don't head-of-line block the wv/wkn loads)
                wo_all = wo_pool.tile([P, MB_HID, NH, P], F16)  # 64 KB/part
                for m in range(MB_HID):
                    nc.gpsimd.dma_start(wo_all[:, m, :, :],
                                        t["w_o_p"][:, m, :, :])
                if with_mask:
                    mask_pool = dctx.enter_context(
                        tc.tile_pool(name="maskp", bufs=4))
                for g in range(NG):
                    # V for the 2 heads of this group: [k, 2*128 dv]
                    if g == 0:
                        wv = wv0
                    else:
                        wv = pdw.tile([P, KB_CKV, 2 * DV], F16, tag="wv")
                        nc.sync.dma_start(wv[:], t["wv_p"][:, g, :, :])
                    v_sb = pdv.tile([P, KB_S, 2 * DV], F16, tag="v")
                    for kb in range(KB_S):
                        psv = psSc.tile([P, W], F32, tag="pss")
                        for kc in range(KB_CKV):
                            _mm(nc, psv[:, :2 * DV],
                                ckT[:, kc, kb * P:(kb + 1) * P],
                                wv[:, kc, :], kc == 0, kc == KB_CKV - 1)
                        nc.scalar.activation(v_sb[:, kb, :],
                                             psv[:, :2 * DV], COPY)

                    for hl in range(2):
                        h = g * 2 + hl
                        # knopeT for head h: [128 d, S]
                        if h < 2:
                            wkn = wkn01[:, h, :, :]
                        else:
                            wkn = pdw.tile([P, KB_CKV, DN], F16, tag="wkn")
                            nc.sync.dma_start(wkn[:], t["wkn_p"][:, h, :, :])
                        knT = pdk.tile([P, NPANEL, W], F16, tag="knT")
                        for nch in range(NPANEL):
                            psk = psSc.tile([P, W], F32, tag="pss")
                            for kc in range(KB_CKV):
                                _mm(nc, psk[:], wkn[:, kc, :],
                                    ckT[:, kc, nch * W:(nch + 1) * W],
                                    kc == 0, kc == KB_CKV - 1)
                            nc.scalar.activation(knT[:, nch, :], psk[:],
                                                 COPY)

                        # attention for head h over all key blocks
                        po = psO.tile([P, W], F32, tag="po")
                        acc = pd.tile([P, W], F16, tag="acc")
                        hp64 = hl * DR
                        for kb in range(KB_S):
                            pss = psSc.tile([P, W], F32, tag="pss")
                            _mm(nc, pss[:],
                                knT[:, kb // 4, (kb % 4) * P:
                                    (kb % 4 + 1) * P],
                                qnopeT[:, h, :], True, False)
                            _mm(nc, pss[:],
                                kpe2[hp64:hp64 + DR, kb * P:(kb + 1) * P],
                                qpeT[hp64:hp64 + DR, g, :], False, True)
                            probs = pprob.tile([P, W], F16, tag="probs")
                            if with_mask:
                                mtile = mask_pool.tile([P, W], F16,
                                                       tag="mt")
                                nc.sync.dma_start(mtile[:],
                                                  t["maskT_p"][:, kb, :])
                                with nc.allow_low_precision(
                                        reason="fp16 probs"):
                                    nc.vector.scalar_tensor_tensor(
                                        probs[:], pss[:], SCALE, mtile[:],
                                        MULT, ADD)
                                nc.scalar.activation(probs[:], probs[:],
                                                     EXP)
                            else:
                                nc.scalar.activation(probs[:], pss[:],
                                                     EXP, scale=SCALE)
                            with nc.allow_low_precision(reason="fp16 acc"):
                                if kb == 0:
                                    nc.vector.tensor_copy(acc[:], probs[:])
                                else:
                                    nc.vector.tensor_tensor(
                                        acc[:], acc[:], probs[:], ADD)
                            _mm(nc, po[:],
                                v_sb[:, kb, hl * DV:(hl + 1) * DV],
                                probs[:], kb == 0, kb == KB_S - 1)
                        sums = pd.tile([P, W], F32, tag="sums")
                        nc.gpsimd.partition_all_reduce(
                            sums[:], acc[:], P, ReduceOp.add)
                        rec = pd.tile([P, W], F32, tag="rec")
                        nc.vector.reciprocal(rec[:], sums[:])
                        with nc.allow_low_precision(reason="fp16 out"):
                            nc.vector.tensor_tensor(oT_sb[:, h, :], po[:],
                                                    rec[:], MULT)

            # ------------- phase E: o_proj (all-resident) ---------------
            pe = pe_pool
            with tc.tile_pool(name="psA", bufs=2, space="PSUM") as psA:
                for m in range(MB_HID):
                    ps = psA.tile([P, W], F32, tag="psA")
                    for k in range(NH):
                        _mm(nc, ps[:], wo_all[:, m, k, :], oT_sb[:, k, :],
                            k == 0, k == NH - 1)
                    osb = pe.tile([P, W], F32, tag="osb")
                    nc.scalar.activation(osb[:], ps[:], COPY)
                    nc.sync.dma_start(t["outT"][m * P:(m + 1) * P, :], osb[:])


def _build_program(with_mask):
    nc = bacc.Bacc("TRN2", target_bir_lowering=False, debug=False,
                   num_devices=NCORES)
    t = {}

    def inp(name, shape, dt=F16):
        t[name] = nc.dram_tensor(name, list(shape), dt,
                                 kind="ExternalInput").ap()

    inp("hsT_panel_p", [P, KB_HID, W])
    inp("w_qa_p", [P, MB_QLR, KB_HID, P])
    inp("w_qb_p", [P, MB_NOPE + MB_PE, KB_QLR, P])
    inp("w_kva_p", [P, KB_HID, KVLR + DR])
    inp("wkn_p", [P, NH, KB_CKV, DN])
    inp("wv_p", [P, NG, KB_CKV, 2 * DV])
    inp("w_o_p", [P, MB_HID, NH, P])
    inp("qa_ln_p", [P, KB_QLR], F32)
    inp("kva_ln_p", [P, KB_CKV], F32)
    inp("cos2p", [P, W], F32)
    inp("sin2sp", [P, W], F32)
    if with_mask:
        inp("maskT_p", [P, KB_S, W])
    t["outT"] = nc.dram_tensor("outT", [HID, W], F32,
                               kind="ExternalOutput").ap()

    with tile.TileContext(nc) as tc:
        _emit(tc, t, with_mask)
    nc.compile()
    return nc


_PROG_CACHE = {}


def _get_program(with_mask):
    if with_mask not in _PROG_CACHE:
        _PROG_CACHE[with_mask] = _build_program(with_mask)
    return _PROG_CACHE[with_mask]


def _pkc(w, kb, mb):
    """[kb*P, mb*P] -> [P, mb, kb, P] host layout (p, m, k, c)."""
    return np.ascontiguousarray(
        np.asarray(w).reshape(kb, P, mb, P).transpose(1, 2, 0, 3)
        .astype(np.float16))


def make_in_maps(hidden_states, attention_mask, cos, sin, w_qa, qa_ln, w_qb,
                 w_kva, kva_ln, w_kvb, w_o, with_mask):
    """Host-side prep: transposes/reorders/fp16 casts; 8 input dicts."""
    f32 = np.float32
    f16 = np.float16
    c = np.ascontiguousarray

    w_qb_r = np.asarray(w_qb).reshape(QLR, NH, DQK)
    w_qb_re = np.concatenate(
        [w_qb_r[:, :, :DN].reshape(QLR, NH * DN),
         w_qb_r[:, :, DN:].reshape(QLR, NH * DR)], axis=1)
    w_kvb_r = np.asarray(w_kvb).reshape(KVLR, NH, DN + DV)
    wkn_p = c(w_kvb_r[:, :, :DN].reshape(KB_CKV, P, NH, DN)
              .transpose(1, 2, 0, 3).astype(f16))
    wv_p = c(w_kvb_r[:, :, DN:].reshape(KVLR, NG, 2 * DV)
             .reshape(KB_CKV, P, NG, 2 * DV).transpose(1, 2, 0, 3)
             .astype(f16))
    qa_ln_p = c(np.asarray(qa_ln).reshape(KB_QLR, P).T.astype(f32))
    kva_ln_p = c(np.asarray(kva_ln).reshape(KB_CKV, P).T.astype(f32))

    cosT = np.asarray(cos).T.astype(f32)                  # [64, S]
    sinT = np.asarray(sin).T.astype(f32)
    sin_s = np.concatenate([-sinT[:DR // 2], sinT[DR // 2:]], axis=0)
    cos2 = np.concatenate([cosT, cosT], axis=0)           # [128, S]
    sin2s = np.concatenate([sin_s, sin_s], axis=0)

    shared = {
        "w_qa_p": _pkc(w_qa, KB_HID, MB_QLR),
        "w_qb_p": _pkc(w_qb_re, KB_QLR, MB_NOPE + MB_PE),
        "w_kva_p": c(np.asarray(w_kva).reshape(KB_HID, P, KVLR + DR)
                     .transpose(1, 0, 2).astype(f16)),
        "wkn_p": wkn_p,
        "wv_p": wv_p,
        "w_o_p": _pkc(w_o, KB_S, MB_HID),
        "qa_ln_p": qa_ln_p,
        "kva_ln_p": kva_ln_p,
    }

    hs = np.asarray(hidden_states)
    am = np.asarray(attention_mask)
    in_maps = []
    for core in range(NCORES):
        b, pnl = divmod(core, NPANEL)
        q0 = pnl * W
        m = dict(shared)
        m["hsT_panel_p"] = c(hs[b, q0:q0 + W, :].T.reshape(KB_HID, P, W)
                             .transpose(1, 0, 2).astype(f16))
        m["cos2p"] = c(cos2[:, q0:q0 + W])
        m["sin2sp"] = c(sin2s[:, q0:q0 + W])
        if with_mask:
            mk = np.maximum(am[b, 0, q0:q0 + W, :].T, -30000.0)  # [S, W]
            m["maskT_p"] = c(mk.reshape(KB_S, P, W).transpose(1, 0, 2)
                             .astype(f16))
        in_maps.append(m)
    return in_maps


def kernel(hidden_states, attention_mask, cos, sin, w_qa, qa_ln, w_qb,
           w_kva, kva_ln, w_kvb, w_o):
    global LAST_RESULT
    with_mask = bool(np.any(np.asarray(attention_mask) != 0))
    nc = _get_program(with_mask)
    in_maps = make_in_maps(hidden_states, attention_mask, cos, sin, w_qa,
                           qa_ln, w_qb, w_kva, kva_ln, w_kvb, w_o, with_mask)
    trace = os.environ.get("KERNEL_TRACE", "0") == "1"
    res = bass_utils.run_bass_kernel_spmd(
        nc, in_maps, core_ids=list(range(NCORES)), trace=trace)
    LAST_RESULT = res

    out = np.empty((B, S, HID), np.float32)
    for core in range(NCORES):
        b, pnl = divmod(core, NPANEL)
        q0 = pnl * W
        out[b, q0:q0 + W, :] = res.results[core]["outT"].T
    return out
